# revision 5
# baseline (speedup 1.0000x reference)
"""Trainium2 Bass kernel for a dense fp32 MultiHeadAttention layer.

Problem (hardcoded): B=4, T=S=2048, C=1024, 16 heads x 64 dims, fp32.
  q = query @ Wq.T + bq ; k,v likewise
  scores = (q k^T) * D**-0.5 + attn_mask + padding_mask
  out = softmax(scores) @ v -> reshape -> @ Wout.T + bout

Sharding over 8 NeuronCores: core c = (batch b = c//2, head-group g = c%2).
Each core handles one batch and 8 of the 16 heads:
  - column-parallel q/k/v projections (512-dim slice of the projections)
  - attention for its 8 heads (full T x S, on-chip scores)
  - row-parallel out_proj producing a partial (T, C) output
Host sums the two partials per batch and adds the bias terms
(bout + bv @ Wout.T, which commutes with softmax since sum(weights)=1).

Layout notes (per core):
  - host ships transposed activations xT (C, T) so projections need no
    on-device transposes:
      qT/kT (f-major): psum = wT_chunk.T @ xT_chunk   (f on partitions)
      v (s-major):     psum = xT_chunk.T @ wT_chunk   (s on partitions)
  - scores are computed transposed, (s on partitions, t free):
      psc = kT_chunk.T @ qT   so softmax's s-reduction becomes a matmul
  - v is stored with a ones column per head (65 wide); the PV matmul
      outT = [v|1].T @ exp(scores^T)
    then yields numerator rows 0..63 and the softmax denominator in row 64.
  - normalization: recip of row 64, broadcast across partitions with a
    one-hot matmul (e1.T @ recip_row), then one DVE multiply.
"""

import os
import numpy as np

import concourse.bass as bass
import concourse.mybir as mybir
import concourse.tile as tile
from concourse import bacc
from concourse.bass_utils import run_bass_kernel_spmd

# ---- problem constants ----
B, T, S, C = 4, 2048, 2048, 1024
H, D = 16, 64
NCORES = 8
F = 512            # per-core projection slice (8 heads x 64)
SCALE = D ** -0.5
P = 128
TCH = 512          # t-chunk (score free dim)
NTC = T // TCH     # 4
NSC = S // P       # 16 s-chunks
NFC = F // P       # 4 f-chunks per core
NCC = C // P       # 8 contraction chunks
HW = 65            # v width per head incl. ones column

FP32 = mybir.dt.float32

# matmul dtype knobs (float32 = exact 4cyc/row, float32r = 1cyc/row reduced precision)
MM_DT = getattr(mybir.dt, os.environ.get("MHA_MM_DT", "float32"))
BC_DT = getattr(mybir.dt, os.environ.get("MHA_BC_DT", "float32"))

LAST_EXEC_NS = None
LAST_TRACE = None
LAST_NC = None
LAST_IN_MAPS = None


def _c(ap, dt):
    return ap if dt == FP32 else ap.bitcast(dt)


def build(use_mask: bool):
    nc = bacc.Bacc("TRN2", target_bir_lowering=False, debug=False,
                   num_devices=NCORES)

    xq = nc.dram_tensor("xq", [C, T], FP32, kind="ExternalInput")
    xk = nc.dram_tensor("xk", [C, S], FP32, kind="ExternalInput")
    xv = nc.dram_tensor("xv", [C, S], FP32, kind="ExternalInput")
    wq = nc.dram_tensor("wq", [C, F], FP32, kind="ExternalInput")
    wk = nc.dram_tensor("wk", [C, F], FP32, kind="ExternalInput")
    wv = nc.dram_tensor("wv", [C, F], FP32, kind="ExternalInput")
    wo = nc.dram_tensor("wo", [F, C], FP32, kind="ExternalInput")
    bqr = nc.dram_tensor("bqr", [P, NFC], FP32, kind="ExternalInput")
    bkr = nc.dram_tensor("bkr", [P, NFC], FP32, kind="ExternalInput")
    if use_mask:
        emask = nc.dram_tensor("emask", [S, T], FP32, kind="ExternalInput")
    out = nc.dram_tensor("out", [T, C], FP32, kind="ExternalOutput")

    xq_r = xq.rearrange("(cc p) t -> p cc t", p=P)
    xk_r = xk.rearrange("(cc p) s -> p cc s", p=P)
    xv_r = xv.rearrange("(cc p) s -> p cc s", p=P)
    wq_r = wq.rearrange("(cc p) f -> p cc f", p=P)
    wk_r = wk.rearrange("(cc p) f -> p cc f", p=P)
    wv_r = wv.rearrange("(cc p) f -> p cc f", p=P)
    wo_r = wo.rearrange("(dc p) f -> p dc f", p=P)

    with tile.TileContext(nc) as tc:
        with tc.tile_pool(name="const", bufs=1) as cp:
            wq_sb = cp.tile([P, NCC, F], FP32, tag="wq")
            wo_sb = cp.tile([P, NFC, C], FP32, tag="wo")
            bq_sb = cp.tile([P, NFC], FP32, tag="bq")
            bk_sb = cp.tile([P, NFC], FP32, tag="bk")
            e1_sb = cp.tile([P, D], BC_DT, tag="e1")
            rrow_sb = cp.tile([P, TCH], BC_DT, tag="rrow")
            kT_sb = cp.tile([P, NFC, S], FP32, tag="kT")
            v_sb = cp.tile([P, NSC, 8 * HW], FP32, tag="v")

            nc.sync.dma_start(wq_sb[:], wq_r[:])
            nc.sync.dma_start(wo_sb[:], wo_r[:])
            nc.sync.dma_start(bq_sb[:], bqr[:])
            nc.sync.dma_start(bk_sb[:], bkr[:])
            nc.any.memset(e1_sb[:], 0.0)
            nc.any.memset(e1_sb[0:1, :], 1.0)
            nc.any.memset(rrow_sb[:], 0.0)
            nc.any.memset(v_sb[:], 1.0)   # ones columns survive at [.., h*65+64]

            # ---------------- phase 1: k/v projections over full S ----------
            with tc.tile_pool(name="ph1w", bufs=1) as wp:
                wk_sb = wp.tile([P, NCC, F], FP32, tag="wk")
                wv_sb = wp.tile([P, NCC, F], FP32, tag="wv")
                nc.sync.dma_start(wk_sb[:], wk_r[:])
                nc.sync.dma_start(wv_sb[:], wv_r[:])
                with (
                    tc.tile_pool(name="ph1s", bufs=3) as sp,
                    tc.tile_pool(name="ph1p", bufs=4, space="PSUM") as pp,
                ):
                    for sw in range(S // TCH):
                        psk = [pp.tile([P, TCH], FP32, tag="psk", name="psk") for _ in range(NFC)]
                        psv = [pp.tile([P, TCH], FP32, tag="psv", name="psv") for _ in range(4)]
                        for cc in range(NCC):
                            xk_t = sp.tile([P, TCH], FP32, tag="xk")
                            xv_t = sp.tile([P, TCH], FP32, tag="xv")
                            nc.sync.dma_start(xk_t[:], xk_r[:, cc, sw * TCH:(sw + 1) * TCH])
                            nc.sync.dma_start(xv_t[:], xv_r[:, cc, sw * TCH:(sw + 1) * TCH])
                            for fc in range(NFC):
                                nc.tensor.matmul(
                                    psk[fc][:],
                                    _c(wk_sb[:, cc, fc * P:(fc + 1) * P], MM_DT),
                                    _c(xk_t[:], MM_DT),
                                    start=(cc == 0), stop=(cc == NCC - 1))
                            for ss in range(4):
                                nc.tensor.matmul(
                                    psv[ss][:],
                                    _c(xv_t[:, ss * P:(ss + 1) * P], MM_DT),
                                    _c(wv_sb[:, cc, :], MM_DT),
                                    start=(cc == 0), stop=(cc == NCC - 1))
                        for fc in range(NFC):
                            nc.vector.tensor_scalar_add(
                                kT_sb[:, fc, sw * TCH:(sw + 1) * TCH],
                                psk[fc][:], bk_sb[:, fc:fc + 1])
                        for ss in range(4):
                            sc = sw * 4 + ss
                            dst = v_sb[:, sc, :].rearrange("p (h e) -> p h e", e=HW)[:, :, 0:D]
                            src = psv[ss][:].rearrange("p (h e) -> p h e", e=D)
                            nc.vector.tensor_copy(dst, src)

            # ---------------- phase 2: main loop over t-chunks --------------
            with (
                tc.tile_pool(name="mainb", bufs=1) as mb_,
                tc.tile_pool(name="mains", bufs=3) as ms,
                tc.tile_pool(name="maino", bufs=2) as mo,
                tc.tile_pool(name="pscore", bufs=2, space="PSUM") as pscp,
                tc.tile_pool(name="ppv", bufs=2, space="PSUM") as ppvp,
                tc.tile_pool(name="pgen", bufs=2, space="PSUM") as pgp,
            ):
                expT = mb_.tile([P, NSC, 2 * TCH], FP32, tag="expT")
                qT_sb = mb_.tile([P, NFC, TCH], FP32, tag="qT")
                attnT = mb_.tile([P, NFC, TCH], FP32, tag="attnT")
                if use_mask:
                    emk_r = emask  # (S, T) natural: s rows

                for tcx in range(NTC):
                    t0 = tcx * TCH
                    # q projection for this t-chunk (2 psum accums at a time)
                    for fcp in range(2):
                        psq = [pgp.tile([P, TCH], FP32, tag="pgen", name="psq") for _ in range(2)]
                        for cc in range(NCC):
                            xq_t = ms.tile([P, TCH], FP32, tag="xq")
                            nc.sync.dma_start(xq_t[:], xq_r[:, cc, t0:t0 + TCH])
                            for i in range(2):
                                fc = fcp * 2 + i
                                nc.tensor.matmul(
                                    psq[i][:],
                                    _c(wq_sb[:, cc, fc * P:(fc + 1) * P], MM_DT),
                                    _c(xq_t[:], MM_DT),
                                    start=(cc == 0), stop=(cc == NCC - 1))
                        for i in range(2):
                            fc = fcp * 2 + i
                            nc.vector.tensor_scalar_add(
                                qT_sb[:, fc, :], psq[i][:], bq_sb[:, fc:fc + 1])

                    for pr in range(NFC):
                        # scores + exp for both heads of the pair
                        for sc in range(NSC):
                            psc = pscp.tile([P, 2, TCH], FP32, tag="pscore")
                            for h in range(2):
                                nc.tensor.matmul(
                                    psc[:, h, :],
                                    _c(kT_sb[h * D:(h + 1) * D, pr, sc * P:(sc + 1) * P], MM_DT),
                                    _c(qT_sb[h * D:(h + 1) * D, pr, :], MM_DT),
                                    start=True, stop=True)
                            nc.scalar.activation(
                                expT[:, sc, :], psc[:].rearrange("p a b -> p (a b)"),
                                mybir.ActivationFunctionType.Exp, scale=SCALE)
                            if use_mask:
                                em_t = ms.tile([P, TCH], FP32, tag="emk")
                                nc.sync.dma_start(
                                    em_t[:], emk_r[sc * P:(sc + 1) * P, t0:t0 + TCH])
                                for h in range(2):
                                    nc.vector.tensor_mul(
                                        expT[:, sc, h * TCH:(h + 1) * TCH],
                                        expT[:, sc, h * TCH:(h + 1) * TCH],
                                        em_t[:])
                        # PV + normalize per head
                        for h in range(2):
                            hh = pr * 2 + h
                            ppv = ppvp.tile([HW, TCH], FP32, tag="ppv")
                            for sc in range(NSC):
                                nc.tensor.matmul(
                                    ppv[:],
                                    _c(v_sb[:, sc, hh * HW:(hh + 1) * HW], MM_DT),
                                    _c(expT[:, sc, h * TCH:(h + 1) * TCH], MM_DT),
                                    start=(sc == 0), stop=(sc == NSC - 1))
                            nc.vector.reciprocal(rrow_sb[0:1, :], ppv[D:D + 1, :])
                            pbc = pgp.tile([D, TCH], FP32, tag="pgen")
                            nc.tensor.matmul(pbc[:], e1_sb[:], rrow_sb[:],
                                             start=True, stop=True)
                            dst = attnT[h * D:(h + 1) * D, pr, :]
                            nc.vector.tensor_copy(dst, ppv[0:D, :])
                            nc.vector.tensor_mul(dst, dst, pbc[:])

                    # out projection for this t-chunk
                    for tw in range(TCH // P):
                        for fh in range(2):
                            po = pgp.tile([P, TCH], FP32, tag="pgen")
                            for dc in range(NFC):
                                nc.tensor.matmul(
                                    po[:],
                                    _c(attnT[:, dc, tw * P:(tw + 1) * P], MM_DT),
                                    _c(wo_sb[:, dc, fh * TCH:(fh + 1) * TCH], MM_DT),
                                    start=(dc == 0), stop=(dc == NFC - 1))
                            ob = mo.tile([P, TCH], FP32, tag="ob")
                            nc.vector.tensor_copy(ob[:], po[:])
                            nc.sync.dma_start(
                                out[t0 + tw * P: t0 + (tw + 1) * P,
                                    fh * TCH:(fh + 1) * TCH],
                                ob[:])

    nc.compile()
    return nc


_CACHE = {}


def _get(use_mask: bool):
    if use_mask not in _CACHE:
        _CACHE[use_mask] = build(use_mask)
    return _CACHE[use_mask]


def kernel(query, key, value, attn_mask, key_padding_mask,
           Wq, bq, Wk, bk, Wv, bv, Wout, bout):
    global LAST_EXEC_NS, LAST_TRACE
    query = np.asarray(query, np.float32)
    key = np.asarray(key, np.float32)
    value = np.asarray(value, np.float32)
    attn_mask = np.asarray(attn_mask, np.float32)
    key_padding_mask = np.asarray(key_padding_mask)
    Wq, bq = np.asarray(Wq, np.float32), np.asarray(bq, np.float32)
    Wk, bk = np.asarray(Wk, np.float32), np.asarray(bk, np.float32)
    Wv, bv = np.asarray(Wv, np.float32), np.asarray(bv, np.float32)
    Wout, bout = np.asarray(Wout, np.float32), np.asarray(bout, np.float32)

    use_mask = bool(np.any(attn_mask)) or bool(np.any(key_padding_mask))
    nc = _get(use_mask)

    in_maps = []
    for c in range(NCORES):
        b, g = divmod(c, 2)
        gs = g * F
        im = {
            "xq": np.ascontiguousarray(query[b].T),
            "xk": np.ascontiguousarray(key[b].T),
            "xv": np.ascontiguousarray(value[b].T),
            "wq": np.ascontiguousarray(Wq[gs:gs + F, :].T),
            "wk": np.ascontiguousarray(Wk[gs:gs + F, :].T),
            "wv": np.ascontiguousarray(Wv[gs:gs + F, :].T),
            "wo": np.ascontiguousarray(Wout[:, gs:gs + F].T),
            "bqr": np.ascontiguousarray(bq[gs:gs + F].reshape(NFC, P).T),
            "bkr": np.ascontiguousarray(bk[gs:gs + F].reshape(NFC, P).T),
        }
        if use_mask:
            m = attn_mask.T.astype(np.float64).copy()
            m[key_padding_mask[b], :] = -np.inf
            im["emask"] = np.exp(m).astype(np.float32)
        in_maps.append(im)

    global LAST_NC, LAST_IN_MAPS
    LAST_NC, LAST_IN_MAPS = nc, in_maps
    res = run_bass_kernel_spmd(nc, in_maps, list(range(NCORES)))
    LAST_EXEC_NS = res.exec_time_ns
    LAST_TRACE = res.instructions_and_trace[1] if res.instructions_and_trace else None

    extra = (bv @ Wout.T + bout).astype(np.float32)
    outp = np.empty((B, T, C), np.float32)
    for b in range(B):
        outp[b] = res.results[2 * b]["out"] + res.results[2 * b + 1]["out"] + extra
    return outp
